# revision 14
# baseline (speedup 1.0000x reference)
"""Encoder-decoder attention kernel for Trainium2, 8 NeuronCores.

Sharding: batch (B=8) data-parallel, one batch element per core; weights
replicated. Per core (S=Sq=Sk=1024, H=1024, NH=16, D=64):

  phase A: transpose X_dec, X_enc via PE -> x_dec_t [h,s], x_enc_t [h,s]
  phase Q: Q^T = (Wq/8) @ X_dec^T (fp32) -> q_t [nd, s]  (upfront, full)
  per head-pair p: K^T pair block JIT (fp32), V 2-pair block JIT
    scores S_n = Q'_n K_n^T (fp32, both heads row-tiled concurrently)
    softmax per q-tile: DVE negated max (one [128,1024] PSUM reduce)
      -> ACT exp(bias=-max, accum_out=rowsum) -> DVE reciprocal
      -> ACT Copy(scale=1/rowsum) normalizing + downcast to fp16
    P^T via ONE hw DMA transpose per tile ([128,1024]->[128,8,128])
    O^T = V^T P^T in fp16 (ev/od heads col-tiled into separate banks)
  phase D: out = concat @ W_out^T + b_out (fp32r matmuls, DVE bias add)

Precision: everything feeding softmax stays fp32 — scores are ~N(0,341)
and exp amplifies score error (fp32r's ~1.5e-4 rounding is too coarse
there). P and V run in fp16 (~2.4e-4; fp16 subnormals cover softmax
tails), the output projection in fp32r. 1/sqrt(D)=1/8 is folded into
W_query on the host (exact in fp32).
"""
import sys

sys.path.insert(0, "/opt/trn_rl_repo")

import numpy as np

B = 8
S = 1024   # Sq == Sk
H = 1024
NH = 16
D = 64
P = 128
HT = H // P    # 8 h-tiles
ST = S // P    # 8 s-tiles == k-tiles
NP = NH // 2   # 8 head pairs
QB = 512       # q-block width for the P@V moving dim
NB = S // QB   # 2 q-blocks
QTB = QB // P  # 4 q-tiles per block


def build():
    import concourse.mybir as mybir
    import concourse.tile as tile
    from concourse import bacc
    from concourse.masks import make_identity

    f32 = mybir.dt.float32
    f32r = mybir.dt.float32r
    f16 = mybir.dt.float16
    bf16 = mybir.dt.bfloat16
    AX = mybir.AxisListType.X
    OP = mybir.AluOpType
    AF = mybir.ActivationFunctionType

    nc = bacc.Bacc(trn_type="TRN2", target_bir_lowering=False, debug=False)

    xd_d = nc.dram_tensor("xd", [S, H], f32, kind="ExternalInput").ap()
    xe_d = nc.dram_tensor("xe", [S, H], f32, kind="ExternalInput").ap()
    wqt_d = nc.dram_tensor("wqt", [H, H], f32, kind="ExternalInput").ap()   # [h, nd] (pre-scaled 1/8)
    wkt_d = nc.dram_tensor("wkt", [H, H], f32, kind="ExternalInput").ap()   # [h, nd]
    wvt_d = nc.dram_tensor("wvt", [H, H], f32, kind="ExternalInput").ap()   # [h, nd]
    wot_d = nc.dram_tensor("wot", [H, H], f32, kind="ExternalInput").ap()   # [nd, h_out]
    bias_d = nc.dram_tensor("bias", [P, H], f32, kind="ExternalInput").ap()
    out_d = nc.dram_tensor("out", [S, H], f32, kind="ExternalOutput").ap()

    from contextlib import ExitStack
    with tile.TileContext(nc) as tc:
        with ExitStack() as ctx:
            big = ctx.enter_context(tc.tile_pool(name="big", bufs=16))
            qtp = ctx.enter_context(tc.tile_pool(name="qt", bufs=2 * NP))
            ksp = ctx.enter_context(tc.tile_pool(name="ks", bufs=8))
            tmpp = ctx.enter_context(tc.tile_pool(name="tmp", bufs=4))
            vpp = ctx.enter_context(tc.tile_pool(name="vp", bufs=2))
            xinp = ctx.enter_context(tc.tile_pool(name="xin", bufs=1))
            pep = ctx.enter_context(tc.tile_pool(name="pe", bufs=2))
            php = ctx.enter_context(tc.tile_pool(name="ph", bufs=2))
            ptp = ctx.enter_context(tc.tile_pool(name="pt", bufs=2))
            wqp = ctx.enter_context(tc.tile_pool(name="wq", bufs=4))
            wvp = ctx.enter_context(tc.tile_pool(name="wv", bufs=2))
            wop = ctx.enter_context(tc.tile_pool(name="wo", bufs=2))
            osbp = ctx.enter_context(tc.tile_pool(name="osb", bufs=1))
            constp = ctx.enter_context(tc.tile_pool(name="const", bufs=1))
            statp = ctx.enter_context(tc.tile_pool(name="stat", bufs=32))
            xehp = ctx.enter_context(tc.tile_pool(name="xeh", bufs=8))
            psp = ctx.enter_context(tc.tile_pool(name="ps", bufs=4, space="PSUM"))
            psSp = ctx.enter_context(tc.tile_pool(name="psS", bufs=2, space="PSUM"))

            def pstile():
                return psp.tile([P, 512], f32, tag="ps", name="ps")

            def pstileS():
                return psSp.tile([P, S], f32, tag="psS", name="psS")

            def stat():
                return statp.tile([P, 1], f32, tag="stat", name="stat")

            # ---- constants ----
            ident = constp.tile([P, P], f32)
            make_identity(nc, ident[:])
            bias_sb = constp.tile([P, H], f32)
            nc.sync.dma_start(bias_sb[:], bias_d)
            # warmup transpose absorbs the gpsimd(identity) dep on PE
            warm = pstile()
            nc.tensor.transpose(warm[:, 0:P], ident[:], ident[:])

            # ---- phase A: X^T via PE transposes ----
            x_dec_t = [big.tile([P, S], f32, tag="big", name="xdt")
                       for _ in range(HT)]
            x_enc_t = [big.tile([P, S], f32, tag="big", name="xet")
                       for _ in range(HT)]
            for lst, src in ((x_dec_t, xd_d), (x_enc_t, xe_d)):
                for i in range(ST):
                    xin = xinp.tile([P, H], f32, tag="xin")
                    nc.sync.dma_start(xin[:], src[i * P:(i + 1) * P, :])
                    for g in range(2):
                        pst = pstile()
                        for t in range(4):
                            j = g * 4 + t
                            nc.tensor.transpose(
                                pst[:, t * P:(t + 1) * P],
                                xin[:, j * P:(j + 1) * P], ident[:])
                        for t in range(4):
                            j = g * 4 + t
                            nc.vector.tensor_copy(
                                lst[j][:, i * P:(i + 1) * P],
                                pst[:, t * P:(t + 1) * P])

            # fp16 copy of X_enc^T for the V projection (V feeds the
            # fp16 P@V path only, so fp16 inputs suffice)
            x_enc_h = [xehp.tile([P, S], f16, tag="xeh", name="xeh")
                       for _ in range(HT)]
            for j in range(HT):
                nc.vector.tensor_copy(x_enc_h[j][:], x_enc_t[j][:])

            # ---- phase Q: full Q^T (fp32): q_t[p] = [128 nd, 1024 s] ----
            q_t = []
            for p in range(NP):
                psq = pstileS()
                for j in range(HT):
                    wt = wqp.tile([P, P], f32, tag="wq")
                    nc.sync.dma_start(
                        wt[:], wqt_d[j * P:(j + 1) * P, p * P:(p + 1) * P])
                    for nn in range(2):
                        nc.tensor.matmul(
                            psq[:, nn * 512:(nn + 1) * 512], wt[:],
                            x_dec_t[j][:, nn * 512:(nn + 1) * 512],
                            start=(j == 0), stop=(j == HT - 1))
                # stacked split tiles: qse = [hi(ev d); lo(ev d)],
                # qso = [hi(od d); lo(od d)] (bf16 hi/lo pairs, K=128)
                qse = qtp.tile([P, S], bf16, tag="qt", name="qse")
                qso = qtp.tile([P, S], bf16, tag="qt", name="qso")
                tmpa = tmpp.tile([P, S], bf16, tag="tmp", name="tmpa")
                tmpb = tmpp.tile([P, S], bf16, tag="tmp", name="tmpb")
                nc.vector.tensor_copy(qse[0:64, :], psq[0:64, :])
                nc.vector.tensor_sub(tmpa[0:64, :], psq[0:64, :], qse[0:64, :])
                nc.vector.tensor_copy(tmpa[64:128, :], psq[64:128, :])
                nc.vector.tensor_sub(tmpb[64:128, :], psq[64:128, :],
                                     tmpa[64:128, :])
                nc.sync.dma_start(qse[64:128, :], tmpa[0:64, :])
                nc.sync.dma_start(qso[0:64, :], tmpa[64:128, :])
                nc.sync.dma_start(qso[64:128, :], tmpb[64:128, :])
                q_t.append((qse, qso))

            # ---- pair loop ----
            concat_t = []
            v2 = None
            for p in range(NP):
                # K^T pair block (fp32): [128 nd, 1024 s]
                psk = pstileS()
                for j in range(HT):
                    wt = wqp.tile([P, P], f32, tag="wq")
                    nc.sync.dma_start(
                        wt[:], wkt_d[j * P:(j + 1) * P, p * P:(p + 1) * P])
                    for nn in range(2):
                        nc.tensor.matmul(
                            psk[:, nn * 512:(nn + 1) * 512], wt[:],
                            x_enc_t[j][:, nn * 512:(nn + 1) * 512],
                            start=(j == 0), stop=(j == HT - 1))
                # ksa_*: [k_hi; k_hi] duplicated across both halves,
                # ksb_*: [k_lo; k_lo] — rhs for the stacked score matmuls
                ksa_ev = ksp.tile([P, S], bf16, tag="ks", name="ksaev")
                ksb_ev = ksp.tile([P, S], bf16, tag="ks", name="ksbev")
                ksa_od = ksp.tile([P, S], bf16, tag="ks", name="ksaod")
                ksb_od = ksp.tile([P, S], bf16, tag="ks", name="ksbod")
                nc.vector.tensor_copy(ksa_ev[0:64, :], psk[0:64, :])
                nc.vector.tensor_sub(ksb_ev[0:64, :], psk[0:64, :],
                                     ksa_ev[0:64, :])
                nc.vector.tensor_copy(ksa_od[64:128, :], psk[64:128, :])
                nc.vector.tensor_sub(ksb_od[64:128, :], psk[64:128, :],
                                     ksa_od[64:128, :])
                nc.sync.dma_start(ksa_ev[64:128, :], ksa_ev[0:64, :])
                nc.sync.dma_start(ksb_ev[64:128, :], ksb_ev[0:64, :])
                nc.sync.dma_start(ksa_od[0:64, :], ksa_od[64:128, :])
                nc.sync.dma_start(ksb_od[0:64, :], ksb_od[64:128, :])
                k_s = ((ksa_ev, ksb_ev), (ksa_od, ksb_od))

                # V block for pairs (p..p+3): v2[k 128, kt, nd 512] fp16
                if p % 4 == 0:
                    v2 = vpp.tile([P, ST, 512], f16, tag="vp")
                    for g in range(2):
                        psv = [pstile() for _ in range(4)]
                        for j in range(HT):
                            wvt_sb = wvp.tile([P, 512], f32, tag="wv")
                            nc.sync.dma_start(
                                wvt_sb[:],
                                wvt_d[j * P:(j + 1) * P, p * P:(p + 4) * P])
                            wvt_h = wvp.tile([P, 512], f16, tag="wvh",
                                             name="wvh")
                            nc.vector.tensor_copy(wvt_h[:], wvt_sb[:])
                            for kk in range(4):
                                kt_i = g * 4 + kk
                                nc.tensor.matmul(
                                    psv[kk][:],
                                    x_enc_h[j][:, kt_i * P:(kt_i + 1) * P],
                                    wvt_h[:],
                                    start=(j == 0), stop=(j == HT - 1))
                        for kk in range(4):
                            nc.vector.tensor_copy(
                                v2[:, g * 4 + kk, :], psv[kk][:])
                vc = (p % 4) * P  # this pair's column base inside v2

                concat = big.tile([P, S], f32r, tag="big", name="concat")
                concat_t.append(concat)

                for blk in range(NB):
                    pt_ev = ptp.tile([P, ST, QB], f16, tag="pt", name="ptev")
                    pt_od = ptp.tile([P, ST, QB], f16, tag="pt", name="ptod")
                    for qtb in range(QTB):
                        qt = blk * QTB + qtb
                        ps_s = [pstileS(), pstileS()]
                        for h01 in range(2):
                            qsplit = q_t[p][h01]
                            ka, kb = k_s[h01]
                            qs = slice(qt * P, (qt + 1) * P)
                            for kk in range(2):
                                ks = slice(kk * 512, (kk + 1) * 512)
                                # S = [qh;ql].T @ [kh;kh] + [qh;ql].T @ [kl;kl]
                                #   = (qh+ql)(kh+kl) exactly
                                nc.tensor.matmul(
                                    ps_s[h01][:, ks], qsplit[:, qs],
                                    ka[:, ks], start=True, stop=False)
                                nc.tensor.matmul(
                                    ps_s[h01][:, ks], qsplit[:, qs],
                                    kb[:, ks], start=False, stop=True)
                        for h01 in range(2):
                            pt_dst = pt_ev if h01 == 0 else pt_od
                            negmax, rsum, recip = stat(), stat(), stat()
                            nc.vector.tensor_reduce(
                                negmax[:], ps_s[h01][:], axis=AX,
                                op=OP.max, negate=True)
                            p_e = pep.tile([P, S], f16, tag="pe")
                            nc.scalar.activation(
                                p_e[:], ps_s[h01][:], AF.Exp,
                                bias=negmax[:], accum_out=rsum[:])
                            nc.vector.reciprocal(recip[:], rsum[:])
                            p_h = php.tile([P, S], f16, tag="ph")
                            nc.scalar.activation(
                                p_h[:], p_e[:], AF.Copy, scale=recip[:])
                            nc.sync.dma_start_transpose(
                                pt_dst[:, :, qtb * P:(qtb + 1) * P], p_h[:])
                    # O^T for the block: [64, QB] per head, fp16, col-tiled
                    ps_oe = pstile()
                    ps_oo = pstile()
                    for kt_i in range(ST):
                        nc.tensor.matmul(
                            ps_oe[0:64, 0:QB],
                            v2[:, kt_i, vc:vc + 64],
                            pt_ev[:, kt_i, :],
                            start=(kt_i == 0), stop=(kt_i == ST - 1),
                            tile_position=(0, 0))
                        nc.tensor.matmul(
                            ps_oo[64:128, 0:QB],
                            v2[:, kt_i, vc + 64:vc + 128],
                            pt_od[:, kt_i, :],
                            start=(kt_i == 0), stop=(kt_i == ST - 1),
                            tile_position=(0, 64))
                    nc.vector.tensor_copy(
                        concat[0:64, blk * QB:(blk + 1) * QB],
                        ps_oe[0:64, 0:QB])
                    nc.vector.tensor_copy(
                        concat[64:128, blk * QB:(blk + 1) * QB],
                        ps_oo[64:128, 0:QB])

            # ---- phase D: out = concat @ W_out^T + b ----
            for sg in range(2):
                ps_out = [pstile(), pstile(), pstile(), pstile(),
                          pstileS(), pstileS()]

                def out_slot(sl, half):
                    # st-local slots 0,1 -> four [128,512] tiles;
                    # slots 2,3 -> halves of two [128,1024] tiles
                    if sl < 2:
                        return ps_out[sl * 2 + half][:]
                    t = ps_out[4 + (sl - 2)]
                    return t[:, half * 512:(half + 1) * 512]

                for p in range(NP):
                    wo_r = []
                    for half in range(2):
                        wo_sb = wop.tile([P, 512], f32, tag="wo")
                        nc.sync.dma_start(
                            wo_sb[:],
                            wot_d[p * P:(p + 1) * P,
                                  half * 512:(half + 1) * 512])
                        wr = wop.tile([P, 512], f32r, tag="wor")
                        nc.vector.tensor_copy(wr[:], wo_sb[:])
                        wo_r.append(wr)
                    for sl in range(4):
                        st = sg * 4 + sl
                        for half in range(2):
                            nc.tensor.matmul(
                                out_slot(sl, half),
                                concat_t[p][:, st * P:(st + 1) * P],
                                wo_r[half][:],
                                start=(p == 0), stop=(p == NP - 1))
                for sl in range(4):
                    st = sg * 4 + sl
                    out_sb = osbp.tile([P, H], f32, tag="osb")
                    for half in range(2):
                        nc.vector.tensor_tensor(
                            out_sb[:, half * 512:(half + 1) * 512],
                            out_slot(sl, half),
                            bias_sb[:, half * 512:(half + 1) * 512],
                            op=OP.add)
                    nc.sync.dma_start(out_d[st * P:(st + 1) * P, :], out_sb[:])

    nc.compile()
    return nc


def prep_in_maps(decoder_input, encoder_output, W_query, W_key, W_value,
                 W_out, b_out):
    f = lambda a: np.ascontiguousarray(np.asarray(a, dtype=np.float32))
    di = f(decoder_input)
    eo = f(encoder_output)
    wq = np.ascontiguousarray((f(W_query).reshape(H, H) * np.float32(0.125)).T)
    wk = np.ascontiguousarray(f(W_key).reshape(H, H).T)
    wv = np.ascontiguousarray(f(W_value).reshape(H, H).T)
    wo = np.ascontiguousarray(f(W_out).T)
    bias = np.ascontiguousarray(np.broadcast_to(f(b_out), (P, H)))
    return [
        {"xd": di[b], "xe": eo[b], "wqt": wq, "wkt": wk, "wvt": wv,
         "wot": wo, "bias": bias}
        for b in range(B)
    ]


_BUILT = None


def kernel(decoder_input, encoder_output, W_query, W_key, W_value, W_out,
           b_out):
    global _BUILT
    from concourse import bass_utils
    if _BUILT is None:
        _BUILT = build()
    in_maps = prep_in_maps(decoder_input, encoder_output, W_query, W_key,
                           W_value, W_out, b_out)
    res = bass_utils.run_bass_kernel_spmd(_BUILT, in_maps,
                                          core_ids=list(range(B)))
    return np.stack([res.results[b]["out"] for b in range(B)], axis=0)
